# revision 1
# baseline (speedup 1.0000x reference)
"""Flipout Bayesian dense layer forward on 8 Trainium2 NeuronCores.

Computes, for x[B,Din], w_loc/w_std/eps_w[Din,Dout], b_loc/b_std[1,Dout],
eps_b[Dout], signs s[B,Din], r1/r2[B,Dout] (all int32 +-1):

    y = x @ w_loc + r1 * ((x*s) @ (softplus(w_std)*eps_w))
        + b_loc + r2 * (softplus(b_std)*eps_b)

Sharding: 4 batch groups x 2 d_out groups across 8 cores. Core c handles
batch rows [(c//2)*1024, ...) and d_out cols [(c%2)*1024, ...). Each core
computes its [1024, 1024] output tile transposed (d_out-major) so the
per-d_out bias terms are per-partition scalars.

All four matmul passes run as fp8e4 DoubleRow (0.5 cyc/row, 256-deep
contraction per instruction), 4x the fp32r row rate:

  p1 = x_hi @ w_hi + x_lo @ w_hi + x_hi @ w_lo     (main, eff. ~2^-8 prec)
  p2 = xs @ ws                                      (perturbation)

with w_hi/w_lo the two-level fp8 split of w_loc*2^WT (host-side),
x_hi/x_lo the split of x at natural scale, xs = fp8(x*s), and
ws = fp8(softplus(w_std)*eps_w*2^WU). Scales make every p1 contribution
uniform at 2^WT so the three passes share one PSUM accumulation chain;
the final ACT copy to bf16 descales by 2^-WT. Measured end-to-end rel
err vs the fp32 reference on the real inputs: 5.4e-3 (gate 2e-2).

Softplus and all operand quantization run on the host, so the device does
no elementwise prep at all: operands DMA straight into SBUF matmul-ready.
Per-core HBM traffic is 16MB vs ~56us of PE time, so the kernel is
PE-bound at the fp8 DoubleRow roofline (TimelineSim ~69.6us/core).

Schedule (all operands SBUF-resident, one explicitly ordered DMA stream):
- Fill: mains m0..3 run pass-major and kp-slab-major so the PE always has
  runnable work while x streams in (pass1 needs only xh, pass2 only xl,
  pass3 is resident). Clumped 512KB slabs (256KB first) keep the p-state
  ramp warm; interleaved stalls at LOW p-state cost 2-4x per row.
- main(m) immediately converts p1 -> t2 = (p1 + z)*2^-WT (DVE+ACT), so
  PSUM banks turn over fast; all psum goes through one shared tag pair.
- Back half weaves pert chains between mains (m4 p0 p1 m5 p2 p3 m6 p4 p5
  p6 m7 p7) so each 5.1us main absorbs two 2.4us pert epilogues; only
  pert7's epilogue trails the last matmul (n-major chain + output halves
  split across both DMA queues keep that tail ~5us).
"""

import numpy as np
import ml_dtypes

import bass_rust as _bass_rust
import concourse.bass as bass
import concourse.tile as tile
from concourse import bacc, mybir
from concourse.bass_utils import run_bass_kernel_spmd
from concourse.hw_specs import get_activation_tables

F32 = mybir.dt.float32
BF16 = mybir.dt.bfloat16
F8 = mybir.dt.float8e4
I8 = mybir.dt.int8
AFT = mybir.ActivationFunctionType
ALU = mybir.AluOpType
DR = mybir.MatmulPerfMode.DoubleRow
E4NP = ml_dtypes.float8_e4m3

D_IN, D_OUT, BATCH = 2048, 2048, 4096
N_CORES = 8
BG, DG = 4, 2                     # batch groups x d_out groups
B_LOC = BATCH // BG               # 1024 batch rows per core
D_LOC = D_OUT // DG               # 1024 d_out cols per core
KT = D_IN // 128                  # 16 k-tiles
KP = KT // 2                      # 8 DoubleRow k-pairs
MT = D_LOC // 128                 # 8 m-tiles (d_out)
NB = B_LOC // 512                 # 2 matmul free-dim chunks of 512

WT = 5                            # w_loc scale 2^WT (fp8 normal range)
WU = 8                            # ws scale 2^WU
PIPE = 3                          # pert/epilogue trail main by PIPE slots

_ONE_TABLE = "natural_log_exp_and_others"

_CACHE = {}


class _Bacc(bacc.Bacc):
    """Bacc that pins every activation to one LUT set (no table thrash)."""

    def insert_act_table_loads(self):
        has_activation = any(
            isinstance(i, mybir.InstActivation)
            for b in self.main_func.blocks
            for i in b.instructions
        )
        if not has_activation:
            return
        all_tables = get_activation_tables(self.m.arch)
        needed = {AFT.Copy, AFT.Identity}
        pinned = all_tables.get(_ONE_TABLE)
        if pinned is not None and needed <= pinned:
            tables = [(name, funcs if name == _ONE_TABLE else set())
                      for name, funcs in all_tables.items()]
        else:
            # fall back to the stock multi-table placement
            tables = list(all_tables.items())
        _bass_rust.insert_act_table_loads(self, tables)


def _build():
    nc = _Bacc("TRN2", target_bir_lowering=False, debug=False)

    # x tensors land as four 512KB slabs of two k-pairs each, [128, 2*2048]
    xh = nc.dram_tensor("xh", [4, 128, 4 * B_LOC], F8, kind="ExternalInput").ap()
    xl = nc.dram_tensor("xl", [4, 128, 4 * B_LOC], F8, kind="ExternalInput").ap()
    xs = nc.dram_tensor("xs", [4, 128, 4 * B_LOC], F8, kind="ExternalInput").ap()
    wh = nc.dram_tensor("wh", [MT, 128, D_IN], F8, kind="ExternalInput").ap()
    wl = nc.dram_tensor("wl", [MT, 128, D_IN], F8, kind="ExternalInput").ap()
    ws = nc.dram_tensor("ws", [MT, 128, D_IN], F8, kind="ExternalInput").ap()
    r1t = nc.dram_tensor("r1t", [MT, 128, B_LOC], I8, kind="ExternalInput").ap()
    r2t = nc.dram_tensor("r2t", [MT, 128, B_LOC], I8, kind="ExternalInput").ap()
    bcols = nc.dram_tensor("bcols", [2, 128, MT], F32, kind="ExternalInput").ap()
    out = nc.dram_tensor("out", [MT, 128, B_LOC], BF16, kind="ExternalOutput").ap()

    with tile.TileContext(nc) as tc:
        with (
            tc.tile_pool(name="xres", bufs=1) as xres,     # resident x fp8 triple
            tc.tile_pool(name="wres", bufs=1) as wres,     # resident w fp8 triple
            tc.tile_pool(name="rres", bufs=1) as rres,     # resident r1/r2 int8
            tc.tile_pool(name="tp", bufs=MT) as tp,        # t2 staging
            tc.tile_pool(name="eo", bufs=2) as eo,         # rf/zt/ob epilogue tiles
            tc.tile_pool(name="bc", bufs=1) as bc,         # bias columns
            tc.tile_pool(name="ps", bufs=4, space="PSUM") as ps,
        ):
            # ---- bias columns: b_loc*2^WT, softplus(b_std)*eps_b*2^WT ----
            blc = bc.tile([128, MT], F32, tag="blc")
            nc.gpsimd.dma_start(blc[:], bcols[0])
            bsm = bc.tile([128, MT], F32, tag="bsm")
            nc.gpsimd.dma_start(bsm[:], bcols[1])

            # ---- resident operand tiles ----
            xht = xres.tile([128, KP, 2, B_LOC], F8, tag="xht")
            xlt = xres.tile([128, KP, 2, B_LOC], F8, tag="xlt")
            xst = xres.tile([128, KP, 2, B_LOC], F8, tag="xst")
            wht = wres.tile([128, MT, KT, 128], F8, tag="wht")
            wlt = wres.tile([128, MT, KT, 128], F8, tag="wlt")
            wst = wres.tile([128, MT, KT, 128], F8, tag="wst")
            r1T = rres.tile([128, MT, B_LOC], I8, tag="r1T")
            r2T = rres.tile([128, MT, B_LOC], I8, tag="r2T")

            # ---- DMA stream, explicitly ordered by first use ----
            # sync/HWDGE queue: 1MB x slabs at full rate, then late w slabs.
            # Pool/SWDGE queue: bias, early w slabs, r tiles, outputs.
            FB = 4                      # fill block: pass-major over m0..3

            def wdma(q, dst, src, m):
                q.dma_start(dst[:, m], src[m])

            def xdma(dst, src, sl):
                nc.sync.dma_start(dst[:, 2 * sl:2 * sl + 2], src[sl])

            # first weight slab and a 256KB first x piece land fastest so
            # the PE starts ~1.3us earlier
            wdma(nc.sync, wht, wh, 0)
            nc.sync.dma_start(xht[:, 0], xh[0][:, 0:2 * B_LOC])
            nc.sync.dma_start(xht[:, 1], xh[0][:, 2 * B_LOC:4 * B_LOC])
            xdma(xht, xh, 1)
            wdma(nc.sync, wht, wh, 1)
            xdma(xht, xh, 2)
            wdma(nc.sync, wht, wh, 2)
            xdma(xht, xh, 3)
            wdma(nc.sync, wht, wh, 3)
            for sl in range(4):
                xdma(xlt, xl, sl)
            for m in range(FB):
                wdma(nc.sync, wlt, wl, m)
            for m in range(FB):
                nc.gpsimd.dma_start(r2T[:, m], r2t[m])
                nc.gpsimd.dma_start(r1T[:, m], r1t[m])
            wdma(nc.sync, wht, wh, 4)
            wdma(nc.sync, wlt, wl, 4)
            xdma(xst, xs, 0)
            xdma(xst, xs, 1)
            wdma(nc.sync, wst, ws, 0)
            wdma(nc.sync, wst, ws, 1)
            xdma(xst, xs, 2)
            wdma(nc.sync, wst, ws, 2)
            xdma(xst, xs, 3)
            wdma(nc.sync, wst, ws, 3)
            wdma(nc.sync, wht, wh, 5)
            wdma(nc.sync, wlt, wl, 5)
            wdma(nc.sync, wht, wh, 6)
            wdma(nc.sync, wlt, wl, 6)
            wdma(nc.sync, wst, ws, 4)
            wdma(nc.sync, wst, ws, 5)
            wdma(nc.sync, wht, wh, 7)
            wdma(nc.sync, wlt, wl, 7)
            wdma(nc.sync, wst, ws, 6)
            wdma(nc.sync, wst, ws, 7)
            for m in range(FB, MT):
                nc.gpsimd.dma_start(r2T[:, m], r2t[m])
                nc.gpsimd.dma_start(r1T[:, m], r1t[m])

            tt = {}    # (m, n) -> t2 tile ((p1 + z) * 2^-WT, awaiting pert)
            p1s = {}   # m -> open p1 psum tile pair

            def alloc_ps(m):
                return [ps.tile([128, 512], F32, tag=f"p1n{n}", name=f"p1n{n}")
                        for n in range(NB)]

            def emit_pass(m, pi, first, last, kps=range(KP)):
                wt_, xt_ = ((wht, xht), (wht, xlt), (wlt, xht))[pi]
                for kp in kps:
                    lw = wt_[:, m, 2 * kp:2 * kp + 2, :]
                    for n in range(NB):
                        nc.tensor.matmul(
                            p1s[m][n][:], lw,
                            xt_[:, kp, :, bass.ts(n, 512)],
                            start=first and kp == 0,
                            stop=last and kp == KP - 1,
                            perf_mode=DR,
                        )

            def emit_tz(m):
                # z = r2*bsamp*2^WT + b_loc*2^WT; t = p1 + z frees PSUM right
                # away; t2 = t * 2^-WT pre-descales off the critical path
                p1 = p1s.pop(m)
                for n in range(NB):
                    zt = eo.tile([128, 512], F32, tag=f"zt{n}")
                    nc.scalar.activation(zt[:], r2T[:, m, bass.ts(n, 512)],
                                         AFT.Identity,
                                         bias=blc[:, m:m + 1],
                                         scale=bsm[:, m:m + 1])
                    t = eo.tile([128, 512], F32, tag=f"t{n}")
                    nc.vector.tensor_tensor(t[:], p1[n][:], zt[:], ALU.add)
                    t2 = tp.tile([128, 512], F32, tag=f"t2n{n}")
                    nc.scalar.activation(t2[:], t[:], AFT.Copy,
                                         scale=float(2.0 ** -WT))
                    tt[(m, n)] = t2

            def emit_main(m):
                p1s[m] = alloc_ps(m)
                for pi in range(3):
                    emit_pass(m, pi, pi == 0, pi == 2)
                emit_tz(m)

            p2s = {}   # m -> open p2 psum tile pair

            def emit_pert_chain(m, kps=range(KP), n_major=False):
                if m not in p2s:
                    p2s[m] = alloc_ps(m)
                p2 = p2s[m]
                order = ([(kp, n) for n in range(NB) for kp in kps] if n_major
                         else [(kp, n) for kp in kps for n in range(NB)])
                for kp, n in order:
                    nc.tensor.matmul(
                        p2[n][:], wst[:, m, 2 * kp:2 * kp + 2, :],
                        xst[:, kp, :, bass.ts(n, 512)],
                        start=kp == 0, stop=kp == KP - 1,
                        perf_mode=DR,
                    )

            def emit_pert_epi(m):
                # y = r1*2^-WU * p2 + t2, finishing in bf16 on DVE
                p2 = p2s.pop(m)
                ob = eo.tile([128, B_LOC], BF16, tag="ob")
                for n in range(NB):
                    rf = eo.tile([128, 512], F32, tag=f"rf{n}")
                    nc.scalar.activation(rf[:], r1T[:, m, bass.ts(n, 512)],
                                         AFT.Copy, scale=float(2.0 ** -WU))
                    nc.vector.tensor_tensor(rf[:], rf[:], p2[n][:], ALU.mult)
                    t2 = tt.pop((m, n))
                    nc.vector.tensor_tensor(ob[:, bass.ts(n, 512)], rf[:],
                                            t2[:], ALU.add)
                    if m == MT - 1:
                        # split the last output across both DMA queues so the
                        # halves issue in parallel off the critical path
                        q = nc.gpsimd if n == 0 else nc.sync
                        q.dma_start(out[m][:, bass.ts(n, 512)],
                                    ob[:, bass.ts(n, 512)])
                if m < MT - 1:
                    nc.gpsimd.dma_start(out[m], ob[:])

            def emit_pert(m, n_major=False):
                emit_pert_chain(m, range(KP), n_major)
                emit_pert_epi(m)

            # ---- fill block: pass-major over m0..FB-1 so the PE always has
            # runnable work while the x tensors stream in (pass1 needs only
            # xh, pass2 only xl, pass3 is fully resident). Slab-major kp
            # order so the in-order PE queue never parks on a late slab
            # while another m's matmuls for the landed slab are ready. ----
            for m in range(FB):
                p1s[m] = alloc_ps(m)
            for pi in range(3):
                for sl in range(4):
                    for m in range(FB):
                        emit_pass(m, pi, pi == 0, pi == 2, range(2 * sl, 2 * sl + 2))
            for m in range(FB):
                emit_tz(m)
            # weave pert chains between the remaining mains: each main gives
            # the DVE 5.1us of slack to drain two pert epilogues, so only
            # pert6/pert7 trail the final main
            emit_main(4)
            emit_pert(0)
            emit_pert(1)
            emit_main(5)
            emit_pert(2)
            emit_pert(3)
            emit_main(6)
            emit_pert(4)
            emit_pert(5)
            emit_pert(6)
            emit_main(7)
            emit_pert(7, n_major=True)

    nc.compile()
    return nc


def _shard(x, w_loc, w_std, b_loc, b_std, eps_w, eps_b, s, r1, r2):
    """Host-side quantization + tiling so every device DMA is contiguous."""
    x = np.asarray(x, dtype=np.float32)
    s_f = np.asarray(s, dtype=np.float32)

    def fp8(a):
        return a.astype(E4NP)

    # two-level fp8 split of x at natural scale
    x_hi = fp8(x)
    x_lo = fp8(x - x_hi.astype(np.float32))
    x_s = fp8(x * s_f)

    # two-level fp8 split of w_loc * 2^WT; ws = softplus(w_std)*eps_w*2^WU
    wp = np.asarray(w_loc, np.float32) * np.float32(2.0 ** WT)
    w_hi = fp8(wp)
    w_lo = fp8(wp - w_hi.astype(np.float32))
    wstd64 = np.asarray(w_std, np.float64)
    wsv = (np.log1p(np.exp(wstd64)).astype(np.float32)
           * np.asarray(eps_w, np.float32)) * np.float32(2.0 ** WU)
    ws8 = fp8(wsv)

    bsamp = (np.log1p(np.exp(np.asarray(b_std, np.float64)[0]))
             .astype(np.float32) * np.asarray(eps_b, np.float32))
    blv = np.asarray(b_loc, np.float32)[0]

    in_maps = []
    for c in range(N_CORES):
        bg, dg = c // DG, c % DG
        rows = slice(bg * B_LOC, (bg + 1) * B_LOC)
        cols = slice(dg * D_LOC, (dg + 1) * D_LOC)

        def wtile(w):
            # [Din, D_LOC] -> [MT, 128, Din]: (m, p=k_in_tile, kt*128+mm)
            w4 = w[:, cols].reshape(KT, 128, MT, 128)
            return np.ascontiguousarray(
                w4.transpose(2, 1, 0, 3).reshape(MT, 128, D_IN))

        def rtile(r):
            # [B_LOC, D_LOC] -> [MT, 128, B_LOC] int8
            return np.ascontiguousarray(
                r[rows][:, cols].T.reshape(MT, 128, B_LOC)).astype(np.int8)

        def ktile(v):
            # [B_LOC, Din] -> [4, 128, 4*B_LOC]: four 512KB slabs of two
            # k-pairs, partition-major within each slab
            vt = v[rows].T.reshape(KT, 128, B_LOC)
            kp8 = (vt.reshape(KP, 2, 128, B_LOC).transpose(0, 2, 1, 3)
                   .reshape(KP, 128, 2 * B_LOC))
            return np.ascontiguousarray(
                kp8.reshape(4, 2, 128, 2 * B_LOC).transpose(0, 2, 1, 3)
                .reshape(4, 128, 4 * B_LOC))

        bpack = np.stack([
            blv[cols].reshape(MT, 128).T * np.float32(2.0 ** WT),
            bsamp[cols].reshape(MT, 128).T * np.float32(2.0 ** WT),
        ]).astype(np.float32)

        in_maps.append(dict(
            xh=ktile(x_hi),
            xl=ktile(x_lo),
            xs=ktile(x_s),
            wh=wtile(w_hi),
            wl=wtile(w_lo),
            ws=wtile(ws8),
            r1t=rtile(np.asarray(r1)),
            r2t=rtile(np.asarray(r2)),
            bcols=np.ascontiguousarray(bpack),
        ))
    return in_maps


def kernel(x, w_loc, w_std, b_loc, b_std, eps_w, eps_b, s, r1, r2, _trace=False):
    if "nc" not in _CACHE:
        _CACHE["nc"] = _build()
    nc = _CACHE["nc"]

    in_maps = _shard(x, w_loc, w_std, b_loc, b_std, eps_w, eps_b, s, r1, r2)
    res = run_bass_kernel_spmd(nc, in_maps, core_ids=list(range(N_CORES)),
                               trace=_trace)

    y = np.empty((BATCH, D_OUT), dtype=np.float32)
    for c in range(N_CORES):
        bg, dg = c // DG, c % DG
        rows = slice(bg * B_LOC, (bg + 1) * B_LOC)
        cols = slice(dg * D_LOC, (dg + 1) * D_LOC)
        o = np.asarray(res.results[c]["out"]).astype(np.float32)
        y[rows, cols] = o.reshape(D_LOC, B_LOC).T
    if _trace:
        return y, res
    return y



# revision 23
# speedup vs baseline: 1.0657x; 1.0657x over previous
"""Flipout Bayesian dense layer forward on 8 Trainium2 NeuronCores.

Computes, for x[B,Din], w_loc/w_std/eps_w[Din,Dout], b_loc/b_std[1,Dout],
eps_b[Dout], signs s[B,Din], r1/r2[B,Dout] (all int32 +-1):

    y = x @ w_loc + r1 * ((x*s) @ (softplus(w_std)*eps_w))
        + b_loc + r2 * (softplus(b_std)*eps_b)

Sharding: 4 batch groups x 2 d_out groups across 8 cores. Core c handles
batch rows [(c//2)*1024, ...) and d_out cols [(c%2)*1024, ...). Each core
computes its [1024, 1024] output tile transposed (d_out-major) so the
per-d_out bias terms are per-partition scalars.

All four matmul passes run as fp8e4 DoubleRow (0.5 cyc/row, 256-deep
contraction per instruction):

  p1 = x_hi @ w_hi + x_hi @ w_lo + x_lo @ w_hi    (main, eff. ~2^-8 prec)
  p2 = xs @ ws                                     (perturbation)

with w_hi/w_lo the two-level fp8 split of w_loc*2^WT (host-side),
x_hi/x_lo the split of x at natural scale, xs = fp8(x*s), and
ws = fp8(softplus(w_std)*eps_w*2^WU). The device output stays at scale
2^WT in bf16 (power-of-two, exact); the host descales during unshard.

Device epilogue is DVE-only on the critical path:
  t[m]  = r2*bsamp*2^WT + b_loc*2^WT   (ACT from int8 r2, precomputed early)
  rf[m] = r1 * 2^(WT-WU)               (ACT from int8 r1, precomputed early)
  t[m] += p1      (DVE, frees PSUM)
  q     = rf * p2 (DVE)
  ob    = t + q   (DVE, bf16)

Schedule: one explicitly ordered DMA stream (sync queue for all inputs,
pool queue for bias + outputs). The PE starts on warmup matmuls over a
memset scratch tile so it is busy (and p-state-ramped) while the first
operands stream in; warmups are also woven into the early fill where the
stream is bandwidth-bound. Fill runs the two xh-passes (x_hi@w_hi,
x_hi@w_lo) for m0-3 in DMA-arrival order, then x_lo@w_hi closing each
accumulation m-by-m so PSUM banks free in main4's alloc order. Back half
weaves pert chains between mains m4..m6, runs pert7 BEFORE main7, and
finishes main7 in shrinking n-chunks (512/256/128/128) so the tail after
the last matmul is two short DVE ops plus one 32KB DMA.
"""

import numpy as np
import ml_dtypes

import bass_rust as _bass_rust
import concourse.bass as bass
import concourse.tile as tile
from concourse import bacc, mybir
from concourse.bass_utils import run_bass_kernel_spmd
from concourse.hw_specs import get_activation_tables

F32 = mybir.dt.float32
BF16 = mybir.dt.bfloat16
F8 = mybir.dt.float8e4
I8 = mybir.dt.int8
AFT = mybir.ActivationFunctionType
ALU = mybir.AluOpType
DR = mybir.MatmulPerfMode.DoubleRow
E4NP = ml_dtypes.float8_e4m3

D_IN, D_OUT, BATCH = 2048, 2048, 4096
N_CORES = 8
BG, DG = 4, 2                     # batch groups x d_out groups
B_LOC = BATCH // BG               # 1024 batch rows per core
D_LOC = D_OUT // DG               # 1024 d_out cols per core
KT = D_IN // 128                  # 16 k-tiles
KP = KT // 2                      # 8 DoubleRow k-pairs
MT = D_LOC // 128                 # 8 m-tiles (d_out)
NB = B_LOC // 512                 # 2 matmul free-dim chunks of 512

WT = 5                            # w_loc scale 2^WT (fp8 normal range)
WU = 8                            # ws scale 2^WU

# warmup matmul counts: head block + gap fillers woven into the early fill
WARM_HEAD = 35
WARM_GAPS = (14, 9)

_ONE_TABLE = "natural_log_exp_and_others"

_CACHE = {}


class _Bacc(bacc.Bacc):
    """Bacc that pins every activation to one LUT set (no table thrash)."""

    def insert_act_table_loads(self):
        has_activation = any(
            isinstance(i, mybir.InstActivation)
            for b in self.main_func.blocks
            for i in b.instructions
        )
        if not has_activation:
            return
        all_tables = get_activation_tables(self.m.arch)
        needed = {AFT.Copy, AFT.Identity}
        pinned = all_tables.get(_ONE_TABLE)
        if pinned is not None and needed <= pinned:
            tables = [(name, funcs if name == _ONE_TABLE else set())
                      for name, funcs in all_tables.items()]
        else:
            # fall back to the stock multi-table placement
            tables = list(all_tables.items())
        _bass_rust.insert_act_table_loads(self, tables)


def _build():
    nc = _Bacc("TRN2", target_bir_lowering=False, debug=False)

    # x tensors: four 512KB slabs of two k-pairs, [slab, 128, kp, 2, B_LOC]
    xh = nc.dram_tensor("xh", [4, 128, 2, 2, B_LOC], F8, kind="ExternalInput").ap()
    xl = nc.dram_tensor("xl", [4, 128, 2, 2, B_LOC], F8, kind="ExternalInput").ap()
    xs = nc.dram_tensor("xs", [4, 128, 2, 2, B_LOC], F8, kind="ExternalInput").ap()
    wh = nc.dram_tensor("wh", [MT, 128, KT, 128], F8, kind="ExternalInput").ap()
    wl = nc.dram_tensor("wl", [MT, 128, KT, 128], F8, kind="ExternalInput").ap()
    ws = nc.dram_tensor("ws", [MT, 128, KT, 128], F8, kind="ExternalInput").ap()
    r1t = nc.dram_tensor("r1t", [MT, 128, B_LOC], I8, kind="ExternalInput").ap()
    r2t = nc.dram_tensor("r2t", [MT, 128, B_LOC], I8, kind="ExternalInput").ap()
    bcols = nc.dram_tensor("bcols", [2, 128, MT], F32, kind="ExternalInput").ap()
    out = nc.dram_tensor("out", [MT, 128, B_LOC], BF16, kind="ExternalOutput").ap()

    with tile.TileContext(nc) as tc:
        with (
            tc.tile_pool(name="xres", bufs=1) as xres,     # resident x fp8 triple
            tc.tile_pool(name="wres", bufs=1) as wres,     # resident w fp8 triple
            tc.tile_pool(name="rres", bufs=1) as rres,     # resident r1/r2 int8
            tc.tile_pool(name="tp", bufs=MT) as tp,        # t = z + p1 staging
            tc.tile_pool(name="rfp", bufs=4) as rfp,       # rf staging
            tc.tile_pool(name="eo", bufs=2) as eo,         # q / ob epilogue tiles
            tc.tile_pool(name="bc", bufs=1) as bc,         # bias columns + scratch
            tc.tile_pool(name="ps", bufs=4, space="PSUM") as ps,
        ):
            # ---- warmup scratch (pool engine); bias comes via sync queue ----
            wsc = bc.tile([128, 2, 256], F8, tag="wsc")
            nc.gpsimd.memset(wsc[:], 0)
            blc = bc.tile([128, MT], F32, tag="blc")
            bsm = bc.tile([128, MT], F32, tag="bsm")

            # ---- resident operand tiles ----
            xht = xres.tile([128, KP, 2, B_LOC], F8, tag="xht")
            xlt = xres.tile([128, KP, 2, B_LOC], F8, tag="xlt")
            xst = xres.tile([128, KP, 2, B_LOC], F8, tag="xst")
            wht = wres.tile([128, MT, KT, 128], F8, tag="wht")
            wlt = wres.tile([128, MT, KT, 128], F8, tag="wlt")
            wst = wres.tile([128, MT, KT, 128], F8, tag="wst")
            r1T = rres.tile([128, MT, B_LOC], I8, tag="r1T")
            r2T = rres.tile([128, MT, B_LOC], I8, tag="r2T")

            # ---- DMA stream on the sync queue, in engine-FIFO order ----
            S = nc.sync

            def wdma(dst, src, m, a=0, b=KT):
                S.dma_start(dst[:, m, a:b], src[m][:, a:b])

            def xdma(dst, src, sl, kp2=None, c0=0, c1=B_LOC):
                if kp2 is None:
                    S.dma_start(dst[:, 2 * sl:2 * sl + 2], src[sl])
                else:
                    S.dma_start(dst[:, 2 * sl + kp2, :, c0:c1],
                                src[sl][:, kp2, :, c0:c1])

            def rdma(dst, src, m):
                S.dma_start(dst[:, m], src[m])

            wdma(wht, wh, 0)              # wh m0 -> first real matmuls
            xdma(xht, xh, 0)              # slab0 (512KB)
            xdma(xht, xh, 1)              # slab1
            wdma(wlt, wl, 0)
            wdma(wht, wh, 1)
            wdma(wlt, wl, 1)
            wdma(wht, wh, 2)
            wdma(wlt, wl, 2)
            wdma(wht, wh, 3)
            wdma(wlt, wl, 3)
            xdma(xlt, xl, 0)
            xdma(xlt, xl, 1)
            xdma(xht, xh, 2)
            xdma(xlt, xl, 2)
            xdma(xht, xh, 3)
            xdma(xlt, xl, 3)
            S.dma_start(blc[:], bcols[0])
            S.dma_start(bsm[:], bcols[1])
            for m in range(4):
                rdma(r2T, r2t, m)
            wdma(wht, wh, 4)
            wdma(wlt, wl, 4)
            xdma(xst, xs, 0)
            wdma(wst, ws, 0)
            xdma(xst, xs, 1)
            xdma(xst, xs, 2)
            xdma(xst, xs, 3)
            wdma(wst, ws, 1)
            wdma(wht, wh, 5)
            wdma(wlt, wl, 5)
            for m in range(4):
                rdma(r1T, r1t, m)
            for m in range(4, MT):
                rdma(r2T, r2t, m)
            wdma(wst, ws, 2)
            wdma(wst, ws, 3)
            wdma(wht, wh, 6)
            wdma(wlt, wl, 6)
            for m in range(4, MT):
                rdma(r1T, r1t, m)
            wdma(wst, ws, 4)
            wdma(wst, ws, 5)
            wdma(wht, wh, 7)
            wdma(wlt, wl, 7)
            wdma(wst, ws, 7)
            wdma(wst, ws, 6)

            # ---- compute streams ----
            tt = {}    # m -> t tile ([128, B_LOC], z then z+p1)
            rfs = {}   # m -> rf tile
            p1s = {}   # m -> open p1 psum pair
            p2s = {}   # m -> open p2 psum pair

            def alloc_ps(m):
                return [ps.tile([128, 512], F32, tag=f"p1n{n}", name=f"p1n{n}")
                        for n in range(NB)]

            # single warmup psum slot: all warmups rewrite it (in-order WAW
            # on the PE needs no semaphores), and it occupies the first p1n0
            # rotation slot so real chains never wait on a warmup.
            wps = ps.tile([128, 512], F32, tag="p1n0", name="wps")

            def warm(k):
                for _ in range(k):
                    nc.tensor.matmul(wps[:, 0:256], wsc[:, :, 0:128], wsc[:],
                                     start=True, stop=True, perf_mode=DR)

            def act_t(m):
                # t = r2*bsamp*2^WT + b_loc*2^WT on ACT, from int8 r2
                t = tp.tile([128, B_LOC], F32, tag="t", name=f"t{m}")
                nc.scalar.activation(t[:], r2T[:, m], AFT.Identity,
                                     bias=blc[:, m:m + 1],
                                     scale=bsm[:, m:m + 1])
                tt[m] = t

            def act_rf(m):
                rf = rfp.tile([128, B_LOC], F32, tag="rf", name=f"rf{m}")
                nc.scalar.activation(rf[:], r1T[:, m], AFT.Copy,
                                     scale=float(2.0 ** (WT - WU)))
                rfs[m] = rf

            def mm(m, pi, kp, n0=0, n1=B_LOC, first=False, last=False,
                   pp=None):
                wt_, xt_ = ((wht, xht), (wht, xlt), (wlt, xht))[pi]
                pt = pp[n0 // 512]
                c0 = n0 % 512
                nc.tensor.matmul(
                    pt[:, c0:c0 + (n1 - n0)],
                    wt_[:, m, 2 * kp:2 * kp + 2, :],
                    xt_[:, kp, :, n0:n1],
                    start=first, stop=last, perf_mode=DR,
                )

            def emit_tadd(m, n):
                # t += p1 on DVE; frees the psum bank
                t = tt[m]
                p1 = p1s[m]
                nc.vector.tensor_tensor(t[:, bass.ts(n, 512)],
                                        t[:, bass.ts(n, 512)],
                                        p1[n][:], ALU.add)

            def emit_pert_chain(m, n_list=None):
                p2s[m] = alloc_ps(m)
                for kp in range(KP):
                    for n in range(NB):
                        nc.tensor.matmul(
                            p2s[m][n][:], wst[:, m, 2 * kp:2 * kp + 2, :],
                            xst[:, kp, :, bass.ts(n, 512)],
                            start=kp == 0, stop=kp == KP - 1, perf_mode=DR,
                        )

            def emit_pert_epi(m, q_only=False):
                # q = rf * p2; ob = t + q (bf16); frees pert psum
                p2 = p2s.pop(m)
                rf = rfs.pop(m)
                if q_only:
                    qs = []
                    for n in range(NB):
                        q = eo.tile([128, 512], F32, tag=f"q{n}",
                                    name=f"q{m}n{n}")
                        nc.vector.tensor_tensor(q[:], rf[:, bass.ts(n, 512)],
                                                p2[n][:], ALU.mult)
                        qs.append(q)
                    return qs
                t = tt.pop(m)
                ob = eo.tile([128, B_LOC], BF16, tag="ob", name=f"ob{m}",
                             bufs=4)
                for n in range(NB):
                    q = eo.tile([128, 512], F32, tag=f"q{n}", name=f"q{m}n{n}")
                    nc.vector.tensor_tensor(q[:], rf[:, bass.ts(n, 512)],
                                            p2[n][:], ALU.mult)
                    # SBUF-only add on the otherwise-idle Pool engine keeps
                    # DVE clear for psum-side work near the tail
                    nc.gpsimd.tensor_tensor(ob[:, bass.ts(n, 512)], q[:],
                                            t[:, bass.ts(n, 512)], ALU.add)
                outq = nc.gpsimd if m < 2 else nc.sync
                outq.dma_start(out[m], ob[:])

            def emit_main(m):
                # full 3-pass main for one m-tile (weave phase); t-adds are
                # emitted separately so the DVE order can favor pert epilogues
                p1s[m] = alloc_ps(m)
                for pi in (0, 2, 1):
                    for kp in range(KP):
                        for n in range(NB):
                            mm(m, pi, kp, n * 512, (n + 1) * 512,
                               first=pi == 0 and kp == 0,
                               last=pi == 1 and kp == KP - 1, pp=p1s[m])

            def emit_tadds(m):
                for n in range(NB):
                    emit_tadd(m, n)
                p1s.pop(m)

            # ===== fill: m0-3, passes (pi0, pi2) in DMA-arrival order, =====
            # ===== then pi1 closing m-by-m; warmups absorb early stalls =====
            for m in range(4):
                p1s[m] = alloc_ps(m)

            def block(m, pi, sls, first=False):
                for sl in sls:
                    for kp in (2 * sl, 2 * sl + 1):
                        for n in range(NB):
                            mm(m, pi, kp, n * 512, (n + 1) * 512,
                               first=first and kp == 0, pp=p1s[m])

            warm(WARM_HEAD)
            block(0, 0, (0,), first=True)
            warm(WARM_GAPS[0])
            block(0, 0, (1,))
            warm(WARM_GAPS[1])
            block(0, 2, (0, 1))
            block(1, 0, (0, 1), first=True)
            block(1, 2, (0, 1))
            block(2, 0, (0, 1), first=True)
            block(2, 2, (0, 1))
            block(3, 0, (0, 1), first=True)
            block(3, 2, (0, 1))
            for sl in (0, 1):
                for m in range(4):
                    block(m, 1, (sl,))
            for m in range(4):
                block(m, 0, (2,))
            for m in range(4):
                block(m, 2, (2,))
            for m in range(4):
                block(m, 1, (2,))
            for m in range(4):
                block(m, 0, (3,))
            for m in range(4):
                block(m, 2, (3,))
            # ACT precompute for m0-3 (r2 + bias landed mid-fill)
            for m in range(4):
                act_t(m)
            # pi1 slab3 closes each accumulation m-by-m, t-adds free PSUM in
            # main4's alloc order
            for m in range(4):
                for kp in (6, 7):
                    for n in range(NB):
                        mm(m, 1, kp, n * 512, (n + 1) * 512,
                           last=kp == 7, pp=p1s[m])
                for n in range(NB):
                    emit_tadd(m, n)
                p1s.pop(m)

            # ===== weave: mains m4-6 absorb pert epilogues; pert7 runs =====
            # ===== before main7 so main7's tail is the short t-add path =====
            act_rf(0)
            act_rf(1)
            act_t(4)
            emit_main(4)
            emit_pert_chain(0)
            emit_pert_epi(0)
            emit_pert_chain(1)
            emit_pert_epi(1)
            emit_tadds(4)
            act_t(5)
            act_rf(2)
            act_rf(3)
            emit_main(5)
            emit_pert_chain(2)
            emit_pert_epi(2)
            emit_pert_chain(3)
            emit_pert_epi(3)
            emit_tadds(5)
            act_t(6)
            act_rf(4)
            act_rf(5)
            emit_main(6)
            emit_pert_chain(4)
            emit_pert_epi(4)
            emit_pert_chain(5)
            emit_pert_epi(5)
            emit_tadds(6)
            act_t(7)
            act_rf(7)
            act_rf(6)
            # pert7 runs BEFORE pert6 so q7 is ready well ahead of main7's
            # chunk epilogues; pert6's ob lands on Pool off the critical path
            emit_pert_chain(7)
            q7 = emit_pert_epi(7, q_only=True)
            emit_pert_chain(6)
            emit_pert_epi(6)

            # ===== main7 in shrinking chunks; per-chunk t-add/ob/DMA =====
            CH = ((0, 256), (256, 256), (512, 256), (768, 128), (896, 128))
            t7 = tt.pop(7)
            ob7 = eo.tile([128, B_LOC], BF16, tag="ob", name="ob7",
                          bufs=4)
            for ci, (c0, w) in enumerate(CH):
                pc = ps.tile([128, 512], F32, tag=f"p1n{ci % 2}",
                             name=f"m7c{ci}")
                for pi in (0, 2, 1):
                    for kp in range(KP):
                        nc.tensor.matmul(
                            pc[:, 0:w],
                            ((wht, wht, wlt)[pi])[:, 7, 2 * kp:2 * kp + 2, :],
                            ((xht, xlt, xht)[pi])[:, kp, :, c0:c0 + w],
                            start=pi == 0 and kp == 0,
                            stop=pi == 1 and kp == KP - 1, perf_mode=DR,
                        )
                n = c0 // 512
                qv = q7[n][:, c0 - n * 512:c0 - n * 512 + w]
                # t7 += p1 on DVE (frees psum); ob = t7 + q is SBUF-only, so
                # the first chunks go to Pool and the last two stay on a
                # by-then-idle DVE for the shortest tail
                nc.vector.tensor_tensor(t7[:, c0:c0 + w], t7[:, c0:c0 + w],
                                        pc[:, 0:w], ALU.add)
                nc.vector.tensor_tensor(ob7[:, c0:c0 + w], t7[:, c0:c0 + w],
                                        qv, ALU.add)
                if ci == 0:
                    nc.gpsimd.dma_start(out[MT - 1][:, c0:c0 + w],
                                        ob7[:, c0:c0 + w])
                elif ci in (1, 2):
                    nc.sync.dma_start(out[MT - 1][:, c0:c0 + w],
                                      ob7[:, c0:c0 + w])
                elif ci == len(CH) - 1:
                    # last two chunks ride one DMA issued after the final ob
                    nc.sync.dma_start(out[MT - 1][:, 768:B_LOC],
                                      ob7[:, 768:B_LOC])

    nc.compile()
    return nc


def _shard(x, w_loc, w_std, b_loc, b_std, eps_w, eps_b, s, r1, r2):
    """Host-side quantization + tiling so every device DMA is contiguous."""
    x = np.asarray(x, dtype=np.float32)
    s_f = np.asarray(s, dtype=np.float32)

    def fp8(a):
        return a.astype(E4NP)

    # two-level fp8 split of x at natural scale
    x_hi = fp8(x)
    x_lo = fp8(x - x_hi.astype(np.float32))
    x_s = fp8(x * s_f)

    # two-level fp8 split of w_loc * 2^WT; ws = softplus(w_std)*eps_w*2^WU
    wp = np.asarray(w_loc, np.float32) * np.float32(2.0 ** WT)
    w_hi = fp8(wp)
    w_lo = fp8(wp - w_hi.astype(np.float32))
    wstd64 = np.asarray(w_std, np.float64)
    wsv = (np.log1p(np.exp(wstd64)).astype(np.float32)
           * np.asarray(eps_w, np.float32)) * np.float32(2.0 ** WU)
    ws8 = fp8(wsv)

    bsamp = (np.log1p(np.exp(np.asarray(b_std, np.float64)[0]))
             .astype(np.float32) * np.asarray(eps_b, np.float32))
    blv = np.asarray(b_loc, np.float32)[0]

    in_maps = []
    for c in range(N_CORES):
        bg, dg = c // DG, c % DG
        rows = slice(bg * B_LOC, (bg + 1) * B_LOC)
        cols = slice(dg * D_LOC, (dg + 1) * D_LOC)

        def wtile(w):
            # [Din, D_LOC] -> [MT, 128, KT, 128]: (m, p=k_in_tile, kt, mm)
            w4 = w[:, cols].reshape(KT, 128, MT, 128)
            return np.ascontiguousarray(
                w4.transpose(2, 1, 0, 3).reshape(MT, 128, KT, 128))

        def rtile(r):
            # [B_LOC, D_LOC] -> [MT, 128, B_LOC] int8
            return np.ascontiguousarray(
                r[rows][:, cols].T.reshape(MT, 128, B_LOC)).astype(np.int8)

        def ktile(v):
            # [B_LOC, Din] -> [4, 128, 2, 2, B_LOC]: four 512KB slabs of two
            # k-pairs, partition-major within each slab
            vt = v[rows].T.reshape(KT, 128, B_LOC)
            kp8 = (vt.reshape(KP, 2, 128, B_LOC).transpose(0, 2, 1, 3)
                   .reshape(KP, 128, 2 * B_LOC))
            return np.ascontiguousarray(
                kp8.reshape(4, 2, 128, 2 * B_LOC).transpose(0, 2, 1, 3)
                .reshape(4, 128, 2, 2, B_LOC))

        bpack = np.stack([
            blv[cols].reshape(MT, 128).T * np.float32(2.0 ** WT),
            bsamp[cols].reshape(MT, 128).T * np.float32(2.0 ** WT),
        ]).astype(np.float32)

        in_maps.append(dict(
            xh=ktile(x_hi),
            xl=ktile(x_lo),
            xs=ktile(x_s),
            wh=wtile(w_hi),
            wl=wtile(w_lo),
            ws=wtile(ws8),
            r1t=rtile(np.asarray(r1)),
            r2t=rtile(np.asarray(r2)),
            bcols=np.ascontiguousarray(bpack),
        ))
    return in_maps


def kernel(x, w_loc, w_std, b_loc, b_std, eps_w, eps_b, s, r1, r2, _trace=False):
    if "nc" not in _CACHE:
        _CACHE["nc"] = _build()
    nc = _CACHE["nc"]

    in_maps = _shard(x, w_loc, w_std, b_loc, b_std, eps_w, eps_b, s, r1, r2)
    res = run_bass_kernel_spmd(nc, in_maps, core_ids=list(range(N_CORES)),
                               trace=_trace)

    y = np.empty((BATCH, D_OUT), dtype=np.float32)
    descale = np.float32(2.0 ** -WT)
    for c in range(N_CORES):
        bg, dg = c // DG, c % DG
        rows = slice(bg * B_LOC, (bg + 1) * B_LOC)
        cols = slice(dg * D_LOC, (dg + 1) * D_LOC)
        o = np.asarray(res.results[c]["out"]).astype(np.float32)
        y[rows, cols] = o.reshape(D_LOC, B_LOC).T * descale
    if _trace:
        return y, res
    return y


# revision 29
# speedup vs baseline: 1.0749x; 1.0086x over previous
"""Flipout Bayesian dense layer forward on 8 Trainium2 NeuronCores.

Computes, for x[B,Din], w_loc/w_std/eps_w[Din,Dout], b_loc/b_std[1,Dout],
eps_b[Dout], signs s[B,Din], r1/r2[B,Dout] (all int32 +-1):

    y = x @ w_loc + r1 * ((x*s) @ (softplus(w_std)*eps_w))
        + b_loc + r2 * (softplus(b_std)*eps_b)

Sharding: 4 batch groups x 2 d_out groups across 8 cores. Core c handles
batch rows [(c//2)*1024, ...) and d_out cols [(c%2)*1024, ...). Each core
computes its [1024, 1024] output tile transposed (d_out-major) so the
per-d_out bias terms are per-partition scalars.

All four matmul passes run as fp8e4 DoubleRow (0.5 cyc/row, 256-deep
contraction per instruction):

  p1 = x_hi @ w_hi + x_hi @ w_lo + x_lo @ w_hi    (main, eff. ~2^-8 prec)
  p2 = xs @ ws                                     (perturbation)

with w_hi/w_lo the two-level fp8 split of w_loc*2^WT (host-side),
x_hi/x_lo the split of x at natural scale, xs = fp8(x*s), and
ws = fp8(softplus(w_std)*eps_w*2^WU). The device output stays at scale
2^WT in bf16 (power-of-two, exact); the host descales during unshard.

Device epilogue is DVE-only on the critical path:
  t[m]  = r2*bsamp*2^WT + b_loc*2^WT   (ACT from int8 r2, precomputed early)
  rf[m] = r1 * 2^(WT-WU)               (ACT from int8 r1, precomputed early)
  t[m] += p1      (DVE, frees PSUM)
  q     = rf * p2 (DVE)
  ob    = t + q   (DVE, bf16)

Schedule: one explicitly ordered DMA stream (sync queue for all inputs,
pool queue for bias + outputs). The PE starts on warmup matmuls over a
memset scratch tile so it is busy (and p-state-ramped) while the first
operands stream in; warmups are also woven into the early fill where the
stream is bandwidth-bound. Fill runs the two xh-passes (x_hi@w_hi,
x_hi@w_lo) for m0-3 in DMA-arrival order, then x_lo@w_hi closing each
accumulation m-by-m so PSUM banks free in main4's alloc order. Back half
weaves pert chains between mains m4..m6, runs pert7 BEFORE main7, and
finishes main7 in shrinking n-chunks (512/256/128/128) so the tail after
the last matmul is two short DVE ops plus one 32KB DMA.
"""

import numpy as np
import ml_dtypes

import bass_rust as _bass_rust
import concourse.bass as bass
import concourse.tile as tile
from concourse import bacc, mybir
from concourse.bass_utils import run_bass_kernel_spmd
from concourse.hw_specs import get_activation_tables

F32 = mybir.dt.float32
BF16 = mybir.dt.bfloat16
F8 = mybir.dt.float8e4
I8 = mybir.dt.int8
AFT = mybir.ActivationFunctionType
ALU = mybir.AluOpType
DR = mybir.MatmulPerfMode.DoubleRow
E4NP = ml_dtypes.float8_e4m3

D_IN, D_OUT, BATCH = 2048, 2048, 4096
N_CORES = 8
BG, DG = 4, 2                     # batch groups x d_out groups
B_LOC = BATCH // BG               # 1024 batch rows per core
D_LOC = D_OUT // DG               # 1024 d_out cols per core
KT = D_IN // 128                  # 16 k-tiles
KP = KT // 2                      # 8 DoubleRow k-pairs
MT = D_LOC // 128                 # 8 m-tiles (d_out)
NB = B_LOC // 512                 # 2 matmul free-dim chunks of 512

WT = 5                            # w_loc scale 2^WT (fp8 normal range)
WU = 8                            # ws scale 2^WU

# warmup matmul counts: head block + gap fillers woven into the early fill
WARM_HEAD = 35
WARM_GAPS = (14, 11)

_ONE_TABLE = "natural_log_exp_and_others"

_CACHE = {}


class _Bacc(bacc.Bacc):
    """Bacc that pins every activation to one LUT set (no table thrash)."""

    def insert_act_table_loads(self):
        has_activation = any(
            isinstance(i, mybir.InstActivation)
            for b in self.main_func.blocks
            for i in b.instructions
        )
        if not has_activation:
            return
        all_tables = get_activation_tables(self.m.arch)
        needed = {AFT.Copy, AFT.Identity}
        pinned = all_tables.get(_ONE_TABLE)
        if pinned is not None and needed <= pinned:
            tables = [(name, funcs if name == _ONE_TABLE else set())
                      for name, funcs in all_tables.items()]
        else:
            # fall back to the stock multi-table placement
            tables = list(all_tables.items())
        _bass_rust.insert_act_table_loads(self, tables)


def _build():
    nc = _Bacc("TRN2", target_bir_lowering=False, debug=False)

    # x tensors: four 512KB slabs of two k-pairs, [slab, 128, kp, 2, B_LOC]
    xh = nc.dram_tensor("xh", [4, 128, 2, 2, B_LOC], F8, kind="ExternalInput").ap()
    xl = nc.dram_tensor("xl", [4, 128, 2, 2, B_LOC], F8, kind="ExternalInput").ap()
    xs = nc.dram_tensor("xs", [4, 128, 2, 2, B_LOC], F8, kind="ExternalInput").ap()
    wh = nc.dram_tensor("wh", [MT, 128, KT, 128], F8, kind="ExternalInput").ap()
    wl = nc.dram_tensor("wl", [MT, 128, KT, 128], F8, kind="ExternalInput").ap()
    ws = nc.dram_tensor("ws", [MT, 128, KT, 128], F8, kind="ExternalInput").ap()
    r1t = nc.dram_tensor("r1t", [MT, 128, B_LOC], I8, kind="ExternalInput").ap()
    r2t = nc.dram_tensor("r2t", [MT, 128, B_LOC], I8, kind="ExternalInput").ap()
    bcols = nc.dram_tensor("bcols", [2, 128, MT], F32, kind="ExternalInput").ap()
    out = nc.dram_tensor("out", [MT, 128, B_LOC], BF16, kind="ExternalOutput").ap()

    with tile.TileContext(nc) as tc:
        with (
            tc.tile_pool(name="xres", bufs=1) as xres,     # resident x fp8 triple
            tc.tile_pool(name="wres", bufs=1) as wres,     # resident w fp8 triple
            tc.tile_pool(name="rres", bufs=1) as rres,     # resident r1/r2 int8
            tc.tile_pool(name="tp", bufs=MT) as tp,        # t = z + p1 staging
            tc.tile_pool(name="rfp", bufs=4) as rfp,       # rf staging
            tc.tile_pool(name="eo", bufs=2) as eo,         # q / ob epilogue tiles
            tc.tile_pool(name="bc", bufs=1) as bc,         # bias columns + scratch
            tc.tile_pool(name="ps", bufs=4, space="PSUM") as ps,
        ):
            # ---- warmup scratch (pool engine); bias comes via sync queue ----
            wsc = bc.tile([128, 2, 256], F8, tag="wsc")
            nc.gpsimd.memset(wsc[:], 0)
            blc = bc.tile([128, MT], F32, tag="blc")
            bsm = bc.tile([128, MT], F32, tag="bsm")

            # ---- resident operand tiles ----
            xht = xres.tile([128, KP, 2, B_LOC], F8, tag="xht")
            xlt = xres.tile([128, KP, 2, B_LOC], F8, tag="xlt")
            xst = xres.tile([128, KP, 2, B_LOC], F8, tag="xst")
            wht = wres.tile([128, MT, KT, 128], F8, tag="wht")
            wlt = wres.tile([128, MT, KT, 128], F8, tag="wlt")
            wst = wres.tile([128, MT, KT, 128], F8, tag="wst")
            r1T = rres.tile([128, MT, B_LOC], I8, tag="r1T")
            r2T = rres.tile([128, MT, B_LOC], I8, tag="r2T")

            # ---- DMA stream on the sync queue, in engine-FIFO order ----
            S = nc.sync

            def wdma(dst, src, m, a=0, b=KT):
                S.dma_start(dst[:, m, a:b], src[m][:, a:b])

            def xdma(dst, src, sl, kp2=None, c0=0, c1=B_LOC):
                if kp2 is None:
                    S.dma_start(dst[:, 2 * sl:2 * sl + 2], src[sl])
                else:
                    S.dma_start(dst[:, 2 * sl + kp2, :, c0:c1],
                                src[sl][:, kp2, :, c0:c1])

            def rdma(dst, src, m):
                S.dma_start(dst[:, m], src[m])

            wdma(wht, wh, 0)              # wh m0 -> first real matmuls
            xdma(xht, xh, 0)              # slab0 (512KB)
            xdma(xht, xh, 1)              # slab1
            wdma(wlt, wl, 0)
            wdma(wht, wh, 1)
            wdma(wlt, wl, 1)
            wdma(wht, wh, 2)
            wdma(wlt, wl, 2)
            wdma(wht, wh, 3)
            wdma(wlt, wl, 3)
            xdma(xlt, xl, 0)
            xdma(xlt, xl, 1)
            xdma(xht, xh, 2)
            xdma(xlt, xl, 2)
            xdma(xht, xh, 3)
            xdma(xlt, xl, 3)
            for m in range(4):
                rdma(r2T, r2t, m)
            S.dma_start(blc[:], bcols[0])
            S.dma_start(bsm[:], bcols[1])
            wdma(wht, wh, 4)
            wdma(wlt, wl, 4)
            xdma(xst, xs, 0)
            wdma(wst, ws, 0)
            xdma(xst, xs, 1)
            xdma(xst, xs, 2)
            xdma(xst, xs, 3)
            wdma(wst, ws, 1)
            wdma(wht, wh, 5)
            wdma(wlt, wl, 5)
            for m in range(4):
                rdma(r1T, r1t, m)
            for m in range(4, MT):
                rdma(r2T, r2t, m)
            wdma(wst, ws, 2)
            wdma(wst, ws, 3)
            wdma(wht, wh, 6)
            wdma(wlt, wl, 6)
            for m in range(4, MT):
                rdma(r1T, r1t, m)
            wdma(wst, ws, 4)
            wdma(wst, ws, 5)
            wdma(wht, wh, 7)
            wdma(wlt, wl, 7)
            wdma(wst, ws, 7)
            wdma(wst, ws, 6)

            # ---- compute streams ----
            tt = {}    # m -> t tile ([128, B_LOC], z then z+p1)
            rfs = {}   # m -> rf tile
            p1s = {}   # m -> open p1 psum pair
            p2s = {}   # m -> open p2 psum pair

            def alloc_ps(m):
                return [ps.tile([128, 512], F32, tag=f"p1n{n}", name=f"p1n{n}")
                        for n in range(NB)]

            # single warmup psum slot: all warmups rewrite it (in-order WAW
            # on the PE needs no semaphores), and it occupies the first p1n0
            # rotation slot so real chains never wait on a warmup.
            wps = ps.tile([128, 512], F32, tag="p1n0", name="wps")

            def warm(k):
                for _ in range(k):
                    nc.tensor.matmul(wps[:, 0:256], wsc[:, :, 0:128], wsc[:],
                                     start=True, stop=True, perf_mode=DR)

            def act_t(m):
                # t = r2*bsamp*2^WT + b_loc*2^WT on ACT, from int8 r2
                t = tp.tile([128, B_LOC], F32, tag="t", name=f"t{m}")
                nc.scalar.activation(t[:], r2T[:, m], AFT.Identity,
                                     bias=blc[:, m:m + 1],
                                     scale=bsm[:, m:m + 1])
                tt[m] = t

            def act_rf(m):
                rf = rfp.tile([128, B_LOC], F32, tag="rf", name=f"rf{m}")
                nc.scalar.activation(rf[:], r1T[:, m], AFT.Copy,
                                     scale=float(2.0 ** (WT - WU)))
                rfs[m] = rf

            def mm(m, pi, kp, n0=0, n1=B_LOC, first=False, last=False,
                   pp=None):
                wt_, xt_ = ((wht, xht), (wht, xlt), (wlt, xht))[pi]
                pt = pp[n0 // 512]
                c0 = n0 % 512
                nc.tensor.matmul(
                    pt[:, c0:c0 + (n1 - n0)],
                    wt_[:, m, 2 * kp:2 * kp + 2, :],
                    xt_[:, kp, :, n0:n1],
                    start=first, stop=last, perf_mode=DR,
                )

            def emit_tadd(m, n):
                # t += p1 on DVE; frees the psum bank
                t = tt[m]
                p1 = p1s[m]
                nc.vector.tensor_tensor(t[:, bass.ts(n, 512)],
                                        t[:, bass.ts(n, 512)],
                                        p1[n][:], ALU.add)

            def emit_pert_chain(m, n_list=None):
                p2s[m] = alloc_ps(m)
                for kp in range(KP):
                    for n in range(NB):
                        nc.tensor.matmul(
                            p2s[m][n][:], wst[:, m, 2 * kp:2 * kp + 2, :],
                            xst[:, kp, :, bass.ts(n, 512)],
                            start=kp == 0, stop=kp == KP - 1, perf_mode=DR,
                        )

            def emit_pert_epi(m, q_only=False):
                # q = rf * p2; ob = t + q (bf16); frees pert psum
                p2 = p2s.pop(m)
                rf = rfs.pop(m)
                if q_only:
                    qs = []
                    for n in range(NB):
                        q = eo.tile([128, 512], F32, tag=f"q{n}",
                                    name=f"q{m}n{n}")
                        nc.vector.tensor_tensor(q[:], rf[:, bass.ts(n, 512)],
                                                p2[n][:], ALU.mult)
                        qs.append(q)
                    return qs
                t = tt.pop(m)
                ob = eo.tile([128, B_LOC], BF16, tag="ob", name=f"ob{m}",
                             bufs=4)
                for n in range(NB):
                    q = eo.tile([128, 512], F32, tag=f"q{n}", name=f"q{m}n{n}")
                    nc.vector.tensor_tensor(q[:], rf[:, bass.ts(n, 512)],
                                            p2[n][:], ALU.mult)
                    # SBUF-only add on the otherwise-idle Pool engine keeps
                    # DVE clear for psum-side work near the tail
                    nc.gpsimd.tensor_tensor(ob[:, bass.ts(n, 512)], q[:],
                                            t[:, bass.ts(n, 512)], ALU.add)
                outq = nc.gpsimd if (m < 2 or m == 6) else nc.sync
                outq.dma_start(out[m], ob[:])

            def emit_main(m):
                # full 3-pass main for one m-tile (weave phase); t-adds are
                # emitted separately so the DVE order can favor pert epilogues
                p1s[m] = alloc_ps(m)
                for pi in (0, 2, 1):
                    for kp in range(KP):
                        for n in range(NB):
                            mm(m, pi, kp, n * 512, (n + 1) * 512,
                               first=pi == 0 and kp == 0,
                               last=pi == 1 and kp == KP - 1, pp=p1s[m])

            def emit_tadds(m):
                for n in range(NB):
                    emit_tadd(m, n)
                p1s.pop(m)

            # ===== fill: m0-3, passes (pi0, pi2) in DMA-arrival order, =====
            # ===== then pi1 closing m-by-m; warmups absorb early stalls =====
            for m in range(4):
                p1s[m] = alloc_ps(m)

            def block(m, pi, sls, first=False):
                for sl in sls:
                    for kp in (2 * sl, 2 * sl + 1):
                        for n in range(NB):
                            mm(m, pi, kp, n * 512, (n + 1) * 512,
                               first=first and kp == 0, pp=p1s[m])

            warm(WARM_HEAD)
            block(0, 0, (0,), first=True)
            warm(WARM_GAPS[0])
            block(0, 0, (1,))
            warm(WARM_GAPS[1])
            block(0, 2, (0, 1))
            block(1, 0, (0, 1), first=True)
            block(1, 2, (0, 1))
            block(2, 0, (0, 1), first=True)
            block(2, 2, (0, 1))
            block(3, 0, (0, 1), first=True)
            block(3, 2, (0, 1))
            for sl in (0, 1):
                for m in range(4):
                    block(m, 1, (sl,))
            for m in range(4):
                block(m, 0, (2,))
            for m in range(4):
                block(m, 2, (2,))
            for m in range(4):
                block(m, 1, (2,))
            for m in range(4):
                block(m, 0, (3,))
            for m in range(4):
                block(m, 2, (3,))
            # ACT precompute for m0-3 (r2 + bias landed mid-fill)
            for m in range(4):
                act_t(m)
            # pi1 slab3 closes each accumulation m-by-m, t-adds free PSUM in
            # main4's alloc order
            for m in range(4):
                for kp in (6, 7):
                    for n in range(NB):
                        mm(m, 1, kp, n * 512, (n + 1) * 512,
                           last=kp == 7, pp=p1s[m])
                for n in range(NB):
                    emit_tadd(m, n)
                p1s.pop(m)

            # ===== weave: mains m4-6 absorb pert epilogues; pert7 runs =====
            # ===== before main7 so main7's tail is the short t-add path =====
            act_rf(0)
            act_rf(1)
            act_t(4)
            emit_main(4)
            emit_pert_chain(0)
            emit_pert_epi(0)
            emit_pert_chain(1)
            emit_pert_epi(1)
            emit_tadds(4)
            act_t(5)
            act_rf(2)
            act_rf(3)
            emit_main(5)
            emit_pert_chain(2)
            emit_pert_epi(2)
            emit_pert_chain(3)
            emit_pert_epi(3)
            emit_tadds(5)
            act_t(6)
            act_rf(4)
            act_rf(5)
            emit_main(6)
            emit_pert_chain(4)
            emit_pert_epi(4)
            emit_pert_chain(5)
            emit_pert_epi(5)
            emit_tadds(6)
            act_t(7)
            act_rf(7)
            act_rf(6)
            # pert7 runs BEFORE pert6 so q7 is ready well ahead of main7's
            # chunk epilogues; pert6's ob lands on Pool off the critical path
            emit_pert_chain(7)
            q7 = emit_pert_epi(7, q_only=True)
            # pre-sum s7 = z7 + q7 on Pool while pert6/main7 run, so each
            # main7 chunk epilogue is a single DVE add from PSUM
            t7 = tt.pop(7)
            for n in range(NB):
                nc.vector.tensor_tensor(t7[:, bass.ts(n, 512)],
                                        t7[:, bass.ts(n, 512)],
                                        q7[n][:], ALU.add)
            emit_pert_chain(6)
            emit_pert_epi(6)

            # ===== main7 in shrinking chunks; per-chunk t-add/ob/DMA =====
            CH = ((0, 256), (256, 256), (512, 256), (768, 128), (896, 128))
            ob7 = eo.tile([128, B_LOC], BF16, tag="ob", name="ob7",
                          bufs=4)
            for ci, (c0, w) in enumerate(CH):
                pc = ps.tile([128, 512], F32, tag=f"p1n{ci % 2}",
                             name=f"m7c{ci}")
                for pi in (0, 2, 1):
                    for kp in range(KP):
                        nc.tensor.matmul(
                            pc[:, 0:w],
                            ((wht, wht, wlt)[pi])[:, 7, 2 * kp:2 * kp + 2, :],
                            ((xht, xlt, xht)[pi])[:, kp, :, c0:c0 + w],
                            start=pi == 0 and kp == 0,
                            stop=pi == 1 and kp == KP - 1, perf_mode=DR,
                        )
                nc.vector.tensor_tensor(ob7[:, c0:c0 + w], pc[:, 0:w],
                                        t7[:, c0:c0 + w], ALU.add)
                if ci in (0, 1, 2):
                    nc.sync.dma_start(out[MT - 1][:, c0:c0 + w],
                                      ob7[:, c0:c0 + w])
                elif ci == len(CH) - 1:
                    # last two chunks ride one DMA issued after the final ob
                    nc.sync.dma_start(out[MT - 1][:, 768:B_LOC],
                                      ob7[:, 768:B_LOC])

    nc.compile()
    return nc


def _shard(x, w_loc, w_std, b_loc, b_std, eps_w, eps_b, s, r1, r2):
    """Host-side quantization + tiling so every device DMA is contiguous."""
    x = np.asarray(x, dtype=np.float32)
    s_f = np.asarray(s, dtype=np.float32)

    def fp8(a):
        return a.astype(E4NP)

    # two-level fp8 split of x at natural scale
    x_hi = fp8(x)
    x_lo = fp8(x - x_hi.astype(np.float32))
    x_s = fp8(x * s_f)

    # two-level fp8 split of w_loc * 2^WT; ws = softplus(w_std)*eps_w*2^WU
    wp = np.asarray(w_loc, np.float32) * np.float32(2.0 ** WT)
    w_hi = fp8(wp)
    w_lo = fp8(wp - w_hi.astype(np.float32))
    wstd64 = np.asarray(w_std, np.float64)
    wsv = (np.log1p(np.exp(wstd64)).astype(np.float32)
           * np.asarray(eps_w, np.float32)) * np.float32(2.0 ** WU)
    ws8 = fp8(wsv)

    bsamp = (np.log1p(np.exp(np.asarray(b_std, np.float64)[0]))
             .astype(np.float32) * np.asarray(eps_b, np.float32))
    blv = np.asarray(b_loc, np.float32)[0]

    in_maps = []
    for c in range(N_CORES):
        bg, dg = c // DG, c % DG
        rows = slice(bg * B_LOC, (bg + 1) * B_LOC)
        cols = slice(dg * D_LOC, (dg + 1) * D_LOC)

        def wtile(w):
            # [Din, D_LOC] -> [MT, 128, KT, 128]: (m, p=k_in_tile, kt, mm)
            w4 = w[:, cols].reshape(KT, 128, MT, 128)
            return np.ascontiguousarray(
                w4.transpose(2, 1, 0, 3).reshape(MT, 128, KT, 128))

        def rtile(r):
            # [B_LOC, D_LOC] -> [MT, 128, B_LOC] int8
            return np.ascontiguousarray(
                r[rows][:, cols].T.reshape(MT, 128, B_LOC)).astype(np.int8)

        def ktile(v):
            # [B_LOC, Din] -> [4, 128, 2, 2, B_LOC]: four 512KB slabs of two
            # k-pairs, partition-major within each slab
            vt = v[rows].T.reshape(KT, 128, B_LOC)
            kp8 = (vt.reshape(KP, 2, 128, B_LOC).transpose(0, 2, 1, 3)
                   .reshape(KP, 128, 2 * B_LOC))
            return np.ascontiguousarray(
                kp8.reshape(4, 2, 128, 2 * B_LOC).transpose(0, 2, 1, 3)
                .reshape(4, 128, 2, 2, B_LOC))

        bpack = np.stack([
            blv[cols].reshape(MT, 128).T * np.float32(2.0 ** WT),
            bsamp[cols].reshape(MT, 128).T * np.float32(2.0 ** WT),
        ]).astype(np.float32)

        in_maps.append(dict(
            xh=ktile(x_hi),
            xl=ktile(x_lo),
            xs=ktile(x_s),
            wh=wtile(w_hi),
            wl=wtile(w_lo),
            ws=wtile(ws8),
            r1t=rtile(np.asarray(r1)),
            r2t=rtile(np.asarray(r2)),
            bcols=np.ascontiguousarray(bpack),
        ))
    return in_maps


def kernel(x, w_loc, w_std, b_loc, b_std, eps_w, eps_b, s, r1, r2, _trace=False):
    if "nc" not in _CACHE:
        _CACHE["nc"] = _build()
    nc = _CACHE["nc"]

    in_maps = _shard(x, w_loc, w_std, b_loc, b_std, eps_w, eps_b, s, r1, r2)
    res = run_bass_kernel_spmd(nc, in_maps, core_ids=list(range(N_CORES)),
                               trace=_trace)

    y = np.empty((BATCH, D_OUT), dtype=np.float32)
    descale = np.float32(2.0 ** -WT)
    for c in range(N_CORES):
        bg, dg = c // DG, c % DG
        rows = slice(bg * B_LOC, (bg + 1) * B_LOC)
        cols = slice(dg * D_LOC, (dg + 1) * D_LOC)
        o = np.asarray(res.results[c]["out"]).astype(np.float32)
        y[rows, cols] = o.reshape(D_LOC, B_LOC).T * descale
    if _trace:
        return y, res
    return y


# revision 31
# speedup vs baseline: 1.0822x; 1.0069x over previous
"""Flipout Bayesian dense layer forward on 8 Trainium2 NeuronCores.

Computes, for x[B,Din], w_loc/w_std/eps_w[Din,Dout], b_loc/b_std[1,Dout],
eps_b[Dout], signs s[B,Din], r1/r2[B,Dout] (all int32 +-1):

    y = x @ w_loc + r1 * ((x*s) @ (softplus(w_std)*eps_w))
        + b_loc + r2 * (softplus(b_std)*eps_b)

Sharding: 4 batch groups x 2 d_out groups across 8 cores. Core c handles
batch rows [(c//2)*1024, ...) and d_out cols [(c%2)*1024, ...). Each core
computes its [1024, 1024] output tile transposed (d_out-major) so the
per-d_out bias terms are per-partition scalars.

All four matmul passes run as fp8e4 DoubleRow (0.5 cyc/row, 256-deep
contraction per instruction):

  p1 = x_hi @ w_hi + x_hi @ w_lo + x_lo @ w_hi    (main, eff. ~2^-8 prec)
  p2 = xs @ ws                                     (perturbation)

with w_hi/w_lo the two-level fp8 split of w_loc*2^WT (host-side),
x_hi/x_lo the split of x at natural scale, xs = fp8(x*s), and
ws = fp8(softplus(w_std)*eps_w*2^WU). The device output stays at scale
2^WT in bf16 (power-of-two, exact); the host descales during unshard.

Device epilogue is DVE-only on the critical path:
  t[m]  = r2*bsamp*2^WT + b_loc*2^WT   (ACT from int8 r2, precomputed early)
  rf[m] = r1 * 2^(WT-WU)               (ACT from int8 r1, precomputed early)
  t[m] += p1      (DVE, frees PSUM)
  q     = rf * p2 (DVE)
  ob    = t + q   (DVE, bf16)

Schedule: one explicitly ordered DMA stream (sync queue for all inputs,
pool queue for bias + outputs). The PE starts on warmup matmuls over a
memset scratch tile so it is busy (and p-state-ramped) while the first
operands stream in; warmups are also woven into the early fill where the
stream is bandwidth-bound. Fill runs the two xh-passes (x_hi@w_hi,
x_hi@w_lo) for m0-3 in DMA-arrival order, then x_lo@w_hi closing each
accumulation m-by-m so PSUM banks free in main4's alloc order. Back half
weaves pert chains between mains m4..m6, runs pert7 BEFORE main7, and
finishes main7 in shrinking n-chunks (512/256/128/128) so the tail after
the last matmul is two short DVE ops plus one 32KB DMA.
"""

import numpy as np
import ml_dtypes

import bass_rust as _bass_rust
import concourse.bass as bass
import concourse.tile as tile
from concourse import bacc, mybir
from concourse.bass_utils import run_bass_kernel_spmd
from concourse.hw_specs import get_activation_tables

F32 = mybir.dt.float32
BF16 = mybir.dt.bfloat16
F8 = mybir.dt.float8e4
I8 = mybir.dt.int8
AFT = mybir.ActivationFunctionType
ALU = mybir.AluOpType
DR = mybir.MatmulPerfMode.DoubleRow
E4NP = ml_dtypes.float8_e4m3

D_IN, D_OUT, BATCH = 2048, 2048, 4096
N_CORES = 8
BG, DG = 4, 2                     # batch groups x d_out groups
B_LOC = BATCH // BG               # 1024 batch rows per core
D_LOC = D_OUT // DG               # 1024 d_out cols per core
KT = D_IN // 128                  # 16 k-tiles
KP = KT // 2                      # 8 DoubleRow k-pairs
MT = D_LOC // 128                 # 8 m-tiles (d_out)
NB = B_LOC // 512                 # 2 matmul free-dim chunks of 512

WT = 5                            # w_loc scale 2^WT (fp8 normal range)
WU = 8                            # ws scale 2^WU

# warmup matmul counts: head block + gap fillers woven into the early fill
WARM_HEAD = 32
WARM_GAPS = (3, 14)

_ONE_TABLE = "natural_log_exp_and_others"

_CACHE = {}


class _Bacc(bacc.Bacc):
    """Bacc that pins every activation to one LUT set (no table thrash)."""

    def insert_act_table_loads(self):
        has_activation = any(
            isinstance(i, mybir.InstActivation)
            for b in self.main_func.blocks
            for i in b.instructions
        )
        if not has_activation:
            return
        all_tables = get_activation_tables(self.m.arch)
        needed = {AFT.Copy, AFT.Identity}
        pinned = all_tables.get(_ONE_TABLE)
        if pinned is not None and needed <= pinned:
            tables = [(name, funcs if name == _ONE_TABLE else set())
                      for name, funcs in all_tables.items()]
        else:
            # fall back to the stock multi-table placement
            tables = list(all_tables.items())
        _bass_rust.insert_act_table_loads(self, tables)


def _build():
    nc = _Bacc("TRN2", target_bir_lowering=False, debug=False)

    # x tensors: four 512KB slabs of two k-pairs, [slab, 128, kp, 2, B_LOC]
    xh = nc.dram_tensor("xh", [4, 128, 2, 2, B_LOC], F8, kind="ExternalInput").ap()
    xl = nc.dram_tensor("xl", [4, 128, 2, 2, B_LOC], F8, kind="ExternalInput").ap()
    xs = nc.dram_tensor("xs", [4, 128, 2, 2, B_LOC], F8, kind="ExternalInput").ap()
    wh = nc.dram_tensor("wh", [MT, 128, KT, 128], F8, kind="ExternalInput").ap()
    wl = nc.dram_tensor("wl", [MT, 128, KT, 128], F8, kind="ExternalInput").ap()
    ws = nc.dram_tensor("ws", [MT, 128, KT, 128], F8, kind="ExternalInput").ap()
    r1t = nc.dram_tensor("r1t", [MT, 128, B_LOC], I8, kind="ExternalInput").ap()
    r2t = nc.dram_tensor("r2t", [MT, 128, B_LOC], I8, kind="ExternalInput").ap()
    bcols = nc.dram_tensor("bcols", [2, 128, MT], F32, kind="ExternalInput").ap()
    out = nc.dram_tensor("out", [MT, 128, B_LOC], BF16, kind="ExternalOutput").ap()

    with tile.TileContext(nc) as tc:
        with (
            tc.tile_pool(name="xres", bufs=1) as xres,     # resident x fp8 triple
            tc.tile_pool(name="wres", bufs=1) as wres,     # resident w fp8 triple
            tc.tile_pool(name="rres", bufs=1) as rres,     # resident r1/r2 int8
            tc.tile_pool(name="tp", bufs=MT) as tp,        # t = z + p1 staging
            tc.tile_pool(name="rfp", bufs=4) as rfp,       # rf staging
            tc.tile_pool(name="eo", bufs=2) as eo,         # q / ob epilogue tiles
            tc.tile_pool(name="bc", bufs=1) as bc,         # bias columns + scratch
            tc.tile_pool(name="ps", bufs=4, space="PSUM") as ps,
        ):
            # ---- warmup scratch (pool engine); bias comes via sync queue ----
            wsc = bc.tile([128, 2, 256], F8, tag="wsc")
            nc.gpsimd.memset(wsc[:], 0)
            blc = bc.tile([128, MT], F32, tag="blc")
            bsm = bc.tile([128, MT], F32, tag="bsm")

            # ---- resident operand tiles ----
            xht = xres.tile([128, KP, 2, B_LOC], F8, tag="xht")
            xlt = xres.tile([128, KP, 2, B_LOC], F8, tag="xlt")
            xst = xres.tile([128, KP, 2, B_LOC], F8, tag="xst")
            wht = wres.tile([128, MT, KT, 128], F8, tag="wht")
            wlt = wres.tile([128, MT, KT, 128], F8, tag="wlt")
            wst = wres.tile([128, MT, KT, 128], F8, tag="wst")
            r1T = rres.tile([128, MT, B_LOC], I8, tag="r1T")
            r2T = rres.tile([128, MT, B_LOC], I8, tag="r2T")

            # ---- DMA stream on the sync queue, in engine-FIFO order ----
            S = nc.sync

            def wdma(dst, src, m, a=0, b=KT):
                S.dma_start(dst[:, m, a:b], src[m][:, a:b])

            def xdma(dst, src, sl, kp2=None, c0=0, c1=B_LOC):
                if kp2 is None:
                    S.dma_start(dst[:, 2 * sl:2 * sl + 2], src[sl])
                else:
                    S.dma_start(dst[:, 2 * sl + kp2, :, c0:c1],
                                src[sl][:, kp2, :, c0:c1])

            def rdma(dst, src, m):
                S.dma_start(dst[:, m], src[m])

            # fill streams w tiles in kt-halves: slabs 0-1 only touch kt0-7,
            # so half-tiles let the PE saturate ~0.75us earlier
            H = KT // 2
            wdma(wht, wh, 0, 0, H)        # wh m0 front half -> first matmuls
            xdma(xht, xh, 0)              # slab0 (512KB)
            wdma(wlt, wl, 0, 0, H)
            xdma(xht, xh, 1)              # slab1
            wdma(wht, wh, 1, 0, H)
            wdma(wlt, wl, 1, 0, H)
            wdma(wht, wh, 2, 0, H)
            wdma(wlt, wl, 2, 0, H)
            wdma(wht, wh, 3, 0, H)
            wdma(wlt, wl, 3, 0, H)
            xdma(xlt, xl, 0)
            xdma(xlt, xl, 1)
            wdma(wht, wh, 0, H, KT)
            xdma(xht, xh, 2)
            wdma(wht, wh, 1, H, KT)
            wdma(wht, wh, 2, H, KT)
            wdma(wht, wh, 3, H, KT)
            wdma(wlt, wl, 0, H, KT)
            wdma(wlt, wl, 1, H, KT)
            xdma(xlt, xl, 2)
            wdma(wlt, wl, 2, H, KT)
            wdma(wlt, wl, 3, H, KT)
            xdma(xht, xh, 3)
            xdma(xlt, xl, 3)
            for m in range(4):
                rdma(r2T, r2t, m)
            S.dma_start(blc[:], bcols[0])
            S.dma_start(bsm[:], bcols[1])
            wdma(wht, wh, 4)
            wdma(wlt, wl, 4)
            xdma(xst, xs, 0)
            wdma(wst, ws, 0)
            xdma(xst, xs, 1)
            xdma(xst, xs, 2)
            xdma(xst, xs, 3)
            wdma(wst, ws, 1)
            wdma(wht, wh, 5)
            wdma(wlt, wl, 5)
            for m in range(4):
                rdma(r1T, r1t, m)
            for m in range(4, MT):
                rdma(r2T, r2t, m)
            wdma(wst, ws, 2)
            wdma(wst, ws, 3)
            wdma(wht, wh, 6)
            wdma(wlt, wl, 6)
            for m in range(4, MT):
                rdma(r1T, r1t, m)
            wdma(wst, ws, 4)
            wdma(wst, ws, 5)
            wdma(wht, wh, 7)
            wdma(wlt, wl, 7)
            wdma(wst, ws, 7)
            wdma(wst, ws, 6)

            # ---- compute streams ----
            tt = {}    # m -> t tile ([128, B_LOC], z then z+p1)
            rfs = {}   # m -> rf tile
            p1s = {}   # m -> open p1 psum pair
            p2s = {}   # m -> open p2 psum pair

            def alloc_ps(m):
                return [ps.tile([128, 512], F32, tag=f"p1n{n}", name=f"p1n{n}")
                        for n in range(NB)]

            # single warmup psum slot: all warmups rewrite it (in-order WAW
            # on the PE needs no semaphores), and it occupies the first p1n0
            # rotation slot so real chains never wait on a warmup.
            wps = ps.tile([128, 512], F32, tag="p1n0", name="wps")

            def warm(k):
                for _ in range(k):
                    nc.tensor.matmul(wps[:, 0:256], wsc[:, :, 0:128], wsc[:],
                                     start=True, stop=True, perf_mode=DR)

            def act_t(m):
                # t = r2*bsamp*2^WT + b_loc*2^WT on ACT, from int8 r2
                t = tp.tile([128, B_LOC], F32, tag="t", name=f"t{m}")
                nc.scalar.activation(t[:], r2T[:, m], AFT.Identity,
                                     bias=blc[:, m:m + 1],
                                     scale=bsm[:, m:m + 1])
                tt[m] = t

            def act_rf(m):
                rf = rfp.tile([128, B_LOC], F32, tag="rf", name=f"rf{m}")
                nc.scalar.activation(rf[:], r1T[:, m], AFT.Copy,
                                     scale=float(2.0 ** (WT - WU)))
                rfs[m] = rf

            def mm(m, pi, kp, n0=0, n1=B_LOC, first=False, last=False,
                   pp=None):
                wt_, xt_ = ((wht, xht), (wht, xlt), (wlt, xht))[pi]
                pt = pp[n0 // 512]
                c0 = n0 % 512
                nc.tensor.matmul(
                    pt[:, c0:c0 + (n1 - n0)],
                    wt_[:, m, 2 * kp:2 * kp + 2, :],
                    xt_[:, kp, :, n0:n1],
                    start=first, stop=last, perf_mode=DR,
                )

            def emit_tadd(m, n):
                # t += p1 on DVE; frees the psum bank
                t = tt[m]
                p1 = p1s[m]
                nc.vector.tensor_tensor(t[:, bass.ts(n, 512)],
                                        t[:, bass.ts(n, 512)],
                                        p1[n][:], ALU.add)

            def emit_pert_chain(m, n_list=None):
                p2s[m] = alloc_ps(m)
                for kp in range(KP):
                    for n in range(NB):
                        nc.tensor.matmul(
                            p2s[m][n][:], wst[:, m, 2 * kp:2 * kp + 2, :],
                            xst[:, kp, :, bass.ts(n, 512)],
                            start=kp == 0, stop=kp == KP - 1, perf_mode=DR,
                        )

            def emit_pert_epi(m, q_only=False):
                # q = rf * p2; ob = t + q (bf16); frees pert psum
                p2 = p2s.pop(m)
                rf = rfs.pop(m)
                if q_only:
                    qs = []
                    for n in range(NB):
                        q = eo.tile([128, 512], F32, tag=f"q{n}",
                                    name=f"q{m}n{n}")
                        nc.vector.tensor_tensor(q[:], rf[:, bass.ts(n, 512)],
                                                p2[n][:], ALU.mult)
                        qs.append(q)
                    return qs
                t = tt.pop(m)
                ob = eo.tile([128, B_LOC], BF16, tag="ob", name=f"ob{m}",
                             bufs=4)
                for n in range(NB):
                    q = eo.tile([128, 512], F32, tag=f"q{n}", name=f"q{m}n{n}")
                    nc.vector.tensor_tensor(q[:], rf[:, bass.ts(n, 512)],
                                            p2[n][:], ALU.mult)
                    # SBUF-only add on the otherwise-idle Pool engine keeps
                    # DVE clear for psum-side work near the tail
                    nc.gpsimd.tensor_tensor(ob[:, bass.ts(n, 512)], q[:],
                                            t[:, bass.ts(n, 512)], ALU.add)
                outq = nc.gpsimd if (m < 2 or m == 6) else nc.sync
                outq.dma_start(out[m], ob[:])

            def emit_main(m):
                # full 3-pass main for one m-tile (weave phase); t-adds are
                # emitted separately so the DVE order can favor pert epilogues
                p1s[m] = alloc_ps(m)
                for pi in (0, 2, 1):
                    for kp in range(KP):
                        for n in range(NB):
                            mm(m, pi, kp, n * 512, (n + 1) * 512,
                               first=pi == 0 and kp == 0,
                               last=pi == 1 and kp == KP - 1, pp=p1s[m])

            def emit_tadds(m):
                for n in range(NB):
                    emit_tadd(m, n)
                p1s.pop(m)

            # ===== fill: m0-3, passes (pi0, pi2) in DMA-arrival order, =====
            # ===== then pi1 closing m-by-m; warmups absorb early stalls =====
            for m in range(4):
                p1s[m] = alloc_ps(m)

            def block(m, pi, sls, first=False):
                for sl in sls:
                    for kp in (2 * sl, 2 * sl + 1):
                        for n in range(NB):
                            mm(m, pi, kp, n * 512, (n + 1) * 512,
                               first=first and kp == 0, pp=p1s[m])

            warm(WARM_HEAD)
            block(0, 0, (0,), first=True)
            warm(WARM_GAPS[0])
            block(0, 2, (0,))
            warm(WARM_GAPS[1])
            block(0, 0, (1,))
            block(0, 2, (1,))
            block(1, 0, (0, 1), first=True)
            block(1, 2, (0, 1))
            block(2, 0, (0, 1), first=True)
            block(2, 2, (0, 1))
            block(3, 0, (0, 1), first=True)
            block(3, 2, (0, 1))
            for sl in (0, 1):
                for m in range(4):
                    block(m, 1, (sl,))
            for m in range(4):
                block(m, 0, (2,))
            for m in range(4):
                block(m, 2, (2,))
            for m in range(4):
                block(m, 1, (2,))
            for m in range(4):
                block(m, 0, (3,))
            for m in range(4):
                block(m, 2, (3,))
            # ACT precompute for m0-3 (r2 + bias landed mid-fill)
            for m in range(4):
                act_t(m)
            # pi1 slab3 closes each accumulation m-by-m, t-adds free PSUM in
            # main4's alloc order
            for m in range(4):
                for kp in (6, 7):
                    for n in range(NB):
                        mm(m, 1, kp, n * 512, (n + 1) * 512,
                           last=kp == 7, pp=p1s[m])
                for n in range(NB):
                    emit_tadd(m, n)
                p1s.pop(m)

            # ===== weave: mains m4-6 absorb pert epilogues; pert7 runs =====
            # ===== before main7 so main7's tail is the short t-add path =====
            act_rf(0)
            act_rf(1)
            act_t(4)
            emit_main(4)
            emit_pert_chain(0)
            emit_pert_epi(0)
            emit_pert_chain(1)
            emit_pert_epi(1)
            emit_tadds(4)
            act_t(5)
            act_rf(2)
            act_rf(3)
            emit_main(5)
            emit_pert_chain(2)
            emit_pert_epi(2)
            emit_pert_chain(3)
            emit_pert_epi(3)
            emit_tadds(5)
            act_t(6)
            act_rf(4)
            act_rf(5)
            emit_main(6)
            emit_pert_chain(4)
            emit_pert_epi(4)
            emit_pert_chain(5)
            emit_pert_epi(5)
            emit_tadds(6)
            act_t(7)
            act_rf(7)
            act_rf(6)
            # pert7 runs BEFORE pert6 so q7 is ready well ahead of main7's
            # chunk epilogues; pert6's ob lands on Pool off the critical path
            emit_pert_chain(7)
            q7 = emit_pert_epi(7, q_only=True)
            # pre-sum s7 = z7 + q7 on Pool while pert6/main7 run, so each
            # main7 chunk epilogue is a single DVE add from PSUM
            t7 = tt.pop(7)
            for n in range(NB):
                nc.vector.tensor_tensor(t7[:, bass.ts(n, 512)],
                                        t7[:, bass.ts(n, 512)],
                                        q7[n][:], ALU.add)
            emit_pert_chain(6)
            emit_pert_epi(6)

            # ===== main7 in shrinking chunks; per-chunk t-add/ob/DMA =====
            CH = ((0, 256), (256, 256), (512, 256), (768, 128), (896, 128))
            ob7 = eo.tile([128, B_LOC], BF16, tag="ob", name="ob7",
                          bufs=4)
            for ci, (c0, w) in enumerate(CH):
                pc = ps.tile([128, 512], F32, tag=f"p1n{ci % 2}",
                             name=f"m7c{ci}")
                for pi in (0, 2, 1):
                    for kp in range(KP):
                        nc.tensor.matmul(
                            pc[:, 0:w],
                            ((wht, wht, wlt)[pi])[:, 7, 2 * kp:2 * kp + 2, :],
                            ((xht, xlt, xht)[pi])[:, kp, :, c0:c0 + w],
                            start=pi == 0 and kp == 0,
                            stop=pi == 1 and kp == KP - 1, perf_mode=DR,
                        )
                nc.vector.tensor_tensor(ob7[:, c0:c0 + w], pc[:, 0:w],
                                        t7[:, c0:c0 + w], ALU.add)
                if ci in (0, 1, 2):
                    nc.sync.dma_start(out[MT - 1][:, c0:c0 + w],
                                      ob7[:, c0:c0 + w])
                elif ci == len(CH) - 1:
                    # last two chunks ride one DMA issued after the final ob
                    nc.sync.dma_start(out[MT - 1][:, 768:B_LOC],
                                      ob7[:, 768:B_LOC])

    nc.compile()
    return nc


def _shard(x, w_loc, w_std, b_loc, b_std, eps_w, eps_b, s, r1, r2):
    """Host-side quantization + tiling so every device DMA is contiguous."""
    x = np.asarray(x, dtype=np.float32)
    s_f = np.asarray(s, dtype=np.float32)

    def fp8(a):
        return a.astype(E4NP)

    # two-level fp8 split of x at natural scale
    x_hi = fp8(x)
    x_lo = fp8(x - x_hi.astype(np.float32))
    x_s = fp8(x * s_f)

    # two-level fp8 split of w_loc * 2^WT; ws = softplus(w_std)*eps_w*2^WU
    wp = np.asarray(w_loc, np.float32) * np.float32(2.0 ** WT)
    w_hi = fp8(wp)
    w_lo = fp8(wp - w_hi.astype(np.float32))
    wstd64 = np.asarray(w_std, np.float64)
    wsv = (np.log1p(np.exp(wstd64)).astype(np.float32)
           * np.asarray(eps_w, np.float32)) * np.float32(2.0 ** WU)
    ws8 = fp8(wsv)

    bsamp = (np.log1p(np.exp(np.asarray(b_std, np.float64)[0]))
             .astype(np.float32) * np.asarray(eps_b, np.float32))
    blv = np.asarray(b_loc, np.float32)[0]

    in_maps = []
    for c in range(N_CORES):
        bg, dg = c // DG, c % DG
        rows = slice(bg * B_LOC, (bg + 1) * B_LOC)
        cols = slice(dg * D_LOC, (dg + 1) * D_LOC)

        def wtile(w):
            # [Din, D_LOC] -> [MT, 128, KT, 128]: (m, p=k_in_tile, kt, mm)
            w4 = w[:, cols].reshape(KT, 128, MT, 128)
            return np.ascontiguousarray(
                w4.transpose(2, 1, 0, 3).reshape(MT, 128, KT, 128))

        def rtile(r):
            # [B_LOC, D_LOC] -> [MT, 128, B_LOC] int8
            return np.ascontiguousarray(
                r[rows][:, cols].T.reshape(MT, 128, B_LOC)).astype(np.int8)

        def ktile(v):
            # [B_LOC, Din] -> [4, 128, 2, 2, B_LOC]: four 512KB slabs of two
            # k-pairs, partition-major within each slab
            vt = v[rows].T.reshape(KT, 128, B_LOC)
            kp8 = (vt.reshape(KP, 2, 128, B_LOC).transpose(0, 2, 1, 3)
                   .reshape(KP, 128, 2 * B_LOC))
            return np.ascontiguousarray(
                kp8.reshape(4, 2, 128, 2 * B_LOC).transpose(0, 2, 1, 3)
                .reshape(4, 128, 2, 2, B_LOC))

        bpack = np.stack([
            blv[cols].reshape(MT, 128).T * np.float32(2.0 ** WT),
            bsamp[cols].reshape(MT, 128).T * np.float32(2.0 ** WT),
        ]).astype(np.float32)

        in_maps.append(dict(
            xh=ktile(x_hi),
            xl=ktile(x_lo),
            xs=ktile(x_s),
            wh=wtile(w_hi),
            wl=wtile(w_lo),
            ws=wtile(ws8),
            r1t=rtile(np.asarray(r1)),
            r2t=rtile(np.asarray(r2)),
            bcols=np.ascontiguousarray(bpack),
        ))
    return in_maps


def kernel(x, w_loc, w_std, b_loc, b_std, eps_w, eps_b, s, r1, r2, _trace=False):
    if "nc" not in _CACHE:
        _CACHE["nc"] = _build()
    nc = _CACHE["nc"]

    in_maps = _shard(x, w_loc, w_std, b_loc, b_std, eps_w, eps_b, s, r1, r2)
    res = run_bass_kernel_spmd(nc, in_maps, core_ids=list(range(N_CORES)),
                               trace=_trace)

    y = np.empty((BATCH, D_OUT), dtype=np.float32)
    descale = np.float32(2.0 ** -WT)
    for c in range(N_CORES):
        bg, dg = c // DG, c % DG
        rows = slice(bg * B_LOC, (bg + 1) * B_LOC)
        cols = slice(dg * D_LOC, (dg + 1) * D_LOC)
        o = np.asarray(res.results[c]["out"]).astype(np.float32)
        y[rows, cols] = o.reshape(D_LOC, B_LOC).T * descale
    if _trace:
        return y, res
    return y
